# revision 39
# baseline (speedup 1.0000x reference)
"""Bass/Trainium2 kernel for nn_DefaultSegmentLinear (fp8 segment linear).

Reference semantics (CHUNKS=4, seg_mode='weight'):
    xq = e4m3fn(x / in_scale)                       # OCP e4m3, max 448
    wq = e4m3fn(w_c / w_scales[c])                  # per out-chunk of 1024
    out = (xq @ wq_c^T) * in_scale * w_scales[c] + bias

Sharding: 8-way over the 16384 tokens (each core owns 2048 tokens and
the full 4096 out features).  Per-core HBM traffic is then 8 MiB of
fp8 x + 16 MiB of fp8 w + 32 MiB of f32 out, far under the tensor
engine's ~445 us of fp8 matmul work, so the kernel is compute-bound.

Quantization runs on the HOST: x and w are divided by their
calibration scales (exact f32 division, matching the reference),
clipped to +-448, halved, and rounded to TRN e4m3 (IEEE-style, max
240).  Every OCP-e4m3 grid point v <= 448 has v/2 exactly
representable in TRN e4m3 (up to deep subnormals), and
round-to-nearest commutes with the exact *0.5, so the device sees
exactly the reference quantization grid at half scale.  The 4x is
folded into the output scale alpha_c = 4*in_scale*w_scales[c].
The device runs pure double-pumped fp8 matmuls (perf_mode=DoubleRow,
K=256 per instruction) with no on-device quantization pass at all.

Per-core tensors (contraction i on partitions for both operands):
    xq4  [16, 128, 2, 2048] fp8  pre-tiled (x/in_scale/2)^T so each
         k-supertile DMA is one 4 KiB contiguous line per partition
    w5d  [32, 128, 16, 2, 128] fp8  pre-tiled (w/w_scale/2)^T so each
         (o-tile, partition) reads 4 KiB contiguous
    outT [4096, 2048] f32  (o, t); host transposes back

PSUM tile [o=128, t=512]; per o-tile: 16 k-steps x 4 t-banks of
DoubleRow matmuls, then one DVE tensor_scalar (psum*alpha + bias) per
bank and a DMA out.  The first two o-tiles interleave their k-loops
(8 matmuls per arriving x k-tile) so the tensor engine keeps pace
with the initial x DMA stream instead of idling at startup; weights
for o-tile n+2 prefetch while n runs.
"""

import os

import ml_dtypes
import numpy as np

import concourse.bacc as bacc
import concourse.mybir as mybir
from concourse import tile
from concourse.bass_utils import run_bass_kernel_spmd

N_CORES = 8
B, S, IN, OUT = 4, 4096, 4096, 4096
TOK = B * S
T = TOK // N_CORES       # 2048 tokens per core
KT = IN // 256           # 16 contraction super-tiles (256 = 128 x 2)
OT = OUT // 128          # 32 out-feature tiles per core
NT = 512                 # moving free dim per matmul (one PSUM bank of f32)
TT = T // NT             # 4 token banks
CHUNKS = 4
OT_PER_CHUNK = OT // CHUNKS  # 8

F32 = mybir.dt.float32
BF16 = mybir.dt.bfloat16
FP8 = mybir.dt.float8e4
E4M3_MAX = 448.0

_CACHE = {}


def _build():
    if "nc" in _CACHE:
        return _CACHE["nc"]
    nc = bacc.Bacc(None, target_bir_lowering=False)
    # x streams as 512 KiB k-tiles with 4 KiB per-partition rows -- the
    # only DMA shape that runs at full ring rate (~386 GB/s; 2 KiB rows
    # measured ~145 GB/s, 1 KiB ~70).  x, the startup weights (wq0/wq1)
    # and the output writes ride the SP HWDGE ring; consts + remaining
    # weights ride the Activation ring, which only sustains ~160 GB/s
    # while SP is busy (HWDGE is FIFO per issuing engine).
    xq4 = nc.dram_tensor("xq4", [KT, 128, 2, T], FP8, kind="ExternalInput")
    w5d = nc.dram_tensor("w5d", [OT, 128, KT, 2, 128], FP8, kind="ExternalInput")
    # cb[p, j] = bias[128*j + p] for j < OT; cb[p, OT+c] = alpha[c].
    # Pre-tiled on host so the whole const set is ONE contiguous DMA --
    # per-element gathers here put ~5k 4-byte packets ahead of the first
    # weight/x tiles on the DMA queue and stall the PE for ~12 us.
    cb = nc.dram_tensor("cb", [128, OT + CHUNKS], F32, kind="ExternalInput")
    outT = nc.dram_tensor("outT", [OUT, T], BF16, kind="ExternalOutput")

    DR = mybir.MatmulPerfMode.DoubleRow

    with tile.TileContext(nc) as tc:
        with (
            tc.tile_pool(name="consts", bufs=1) as consts,
            tc.tile_pool(name="xq", bufs=1) as xqp,
            tc.tile_pool(name="wq", bufs=4) as wqp,
            tc.tile_pool(name="osb", bufs=4) as osbp,
            tc.tile_pool(name="psum", bufs=8, space="PSUM") as psp,
        ):
            wq_t = {}

            def load_wq(ot, eng):
                t = wqp.tile([128, KT, 2, 128], FP8, tag="wq", name=f"wq{ot}")
                eng.dma_start(out=t[:], in_=w5d[ot])
                wq_t[ot] = t

            # Each dma_start carries ~2 us of completion latency on top of
            # its transfer time, so the two first-matmul dependencies (xq0,
            # wq0) must ride DIFFERENT rings in parallel, each as its
            # ring's first item: x (then the output writes) on SP, weights
            # + consts on ACT.  Steady-state weight prefetches stay on ACT,
            # paced by wq-pool buffer reuse.
            xq = []

            def load_xq(k):
                xq_k = xqp.tile([128, 2, T], FP8, tag=f"xq{k}", name=f"xq{k}")
                nc.sync.dma_start(out=xq_k[:], in_=xq4[k])
                xq.append(xq_k)

            load_wq(0, nc.scalar)
            for k in range(KT):
                load_xq(k)
            load_wq(1, nc.scalar)
            cb_sb = consts.tile([128, OT + CHUNKS], F32, tag="cb")
            nc.scalar.dma_start(out=cb_sb[:], in_=cb[:])
            load_wq(2, nc.scalar)
            load_wq(3, nc.scalar)

            def rhs(k, tb):
                return xq[k][:, :, NT * tb : NT * (tb + 1)]

            def mms(wq, ps, k):
                for tb in range(TT):
                    nc.tensor.matmul(
                        ps[tb][:],
                        lhsT=wq[:, k, :, :],
                        rhs=rhs(k, tb),
                        start=(k == 0),
                        stop=(k == KT - 1),
                        perf_mode=DR,
                    )

            # One [128, T] bf16 out tile per o-tile: 4 banks of DVE
            # scale+bias land in its columns, then a single DMA with 4 KiB
            # rows writes it out (small-row DMAs run far below ring rate).
            def epilogue(ot, ps):
                c = ot // OT_PER_CHUNK
                ob = osbp.tile([128, T], BF16, tag="osb", name=f"ob{ot}")
                for tb in range(TT):
                    nc.vector.tensor_scalar(
                        ob[:, NT * tb : NT * (tb + 1)],
                        ps[tb][:],
                        cb_sb[:, OT + c : OT + c + 1],
                        cb_sb[:, ot : ot + 1],
                        op0=mybir.AluOpType.mult,
                        op1=mybir.AluOpType.add,
                    )
                nc.sync.dma_start(
                    out=outT[128 * ot : 128 * (ot + 1), :], in_=ob[:]
                )

            # ---- startup: o-tiles 0,1 with k-interleaved loops, so each
            # arriving xq[k] feeds 8 matmuls and the PE tracks the DMA
            # frontier ----
            psA = [
                [
                    psp.tile([128, NT], F32, tag="ps", name=f"ps{ot}_{tb}")
                    for tb in range(TT)
                ]
                for ot in range(2)
            ]
            # ot0 runs k0..k2 solo first: at ramp clock its consumption
            # matches the x delivery rate, and it buys time for wq1 to
            # arrive on the ACT ring (~16 us) without stalling the PE.
            # ot1 then catches up from resident tiles before the two
            # interleave for the rest of the k range.
            SOLO = 4
            for k in range(SOLO):
                mms(wq_t[0], psA[0], k)
            for k in range(SOLO):
                mms(wq_t[1], psA[1], k)
            for k in range(SOLO, KT):
                for h in range(2):
                    mms(wq_t[h], psA[h], k)
            for h in range(2):
                epilogue(h, psA[h])

            # ---- steady state: stream o-tiles 2..31, prefetch depth 2 ----
            for ot in range(2, OT - 1):
                ps = [
                    psp.tile([128, NT], F32, tag="ps", name=f"ps{ot}_{tb}")
                    for tb in range(TT)
                ]
                for k in range(KT):
                    mms(wq_t[ot], ps, k)
                if ot + 2 < OT:
                    load_wq(ot + 2, nc.scalar)
                epilogue(ot, ps)

            # ---- last o-tile: bank-by-bank (k inner) so each bank's
            # epilogue overlaps the remaining banks' matmuls ----
            ot = OT - 1
            c = ot // OT_PER_CHUNK
            obL = osbp.tile([128, T], BF16, tag="osb", name=f"ob{ot}")
            for tb in range(TT):
                ps_tb = psp.tile([128, NT], F32, tag="ps", name=f"ps{ot}_{tb}")
                for k in range(KT):
                    nc.tensor.matmul(
                        ps_tb[:],
                        lhsT=wq_t[ot][:, k, :, :],
                        rhs=rhs(k, tb),
                        start=(k == 0),
                        stop=(k == KT - 1),
                        perf_mode=DR,
                    )
                nc.vector.tensor_scalar(
                    obL[:, NT * tb : NT * (tb + 1)],
                    ps_tb[:],
                    cb_sb[:, OT + c : OT + c + 1],
                    cb_sb[:, ot : ot + 1],
                    op0=mybir.AluOpType.mult,
                    op1=mybir.AluOpType.add,
                )
            nc.sync.dma_start(out=outT[128 * ot : 128 * (ot + 1), :], in_=obL[:])
    nc.compile()
    _CACHE["nc"] = nc
    return nc


def prepare_in_maps(x, w, bias, in_scale, w_scales):
    """Host-side prep: scale-normalize, quantize to TRN e4m3, tile.

    Matches the reference grid: v = x/in_scale (exact f32 division),
    clip to +-448, then round-to-nearest onto the OCP e4m3 grid -- done
    here as round(v*0.5) onto the TRN e4m3 grid, identical because the
    grids coincide under the exact *0.5 (up to deep subnormals below
    2^-9, which are noise at this problem's scale).
    """
    assert x.shape == (B, S, IN) and w.shape == (OUT, IN)
    x = np.ascontiguousarray(x, dtype=np.float32)
    w = np.ascontiguousarray(w, dtype=np.float32)
    bias = np.ascontiguousarray(bias, dtype=np.float32)
    in_scale = np.float32(np.asarray(in_scale).reshape(()))
    w_scales = np.asarray(w_scales, dtype=np.float32).reshape(CHUNKS)

    t = x.reshape(TOK, IN) / in_scale
    np.clip(t, -E4M3_MAX, E4M3_MAX, out=t)
    t *= np.float32(0.5)
    xq8 = t.astype(ml_dtypes.float8_e4m3)
    # xq4[q][k, p, j, t'] = xq8[t = T*q + t', i = 256k + 128j + p]
    xq4 = np.ascontiguousarray(
        xq8.reshape(N_CORES, T, KT, 2, 128).transpose(0, 2, 4, 3, 1)
    )

    wn = w.reshape(CHUNKS, OUT // CHUNKS, IN) / w_scales[:, None, None]
    np.clip(wn, -E4M3_MAX, E4M3_MAX, out=wn)
    wn *= np.float32(0.5)
    wq8 = wn.reshape(OUT, IN).astype(ml_dtypes.float8_e4m3)
    # w5d[ot, p, k, j, m] = wq8[o = 128*ot + m, i = 256k + 128j + p]
    w5d = np.ascontiguousarray(
        wq8.T.reshape(KT, 2, 128, OT, 128).transpose(3, 2, 0, 1, 4)
    )

    alpha = (
        4.0 * in_scale.astype(np.float64) * w_scales.astype(np.float64)
    ).astype(np.float32)
    # cb[p, j<OT] = bias[128*j + p]; cb[p, OT+c] = alpha[c]
    cb = np.empty((128, OT + CHUNKS), dtype=np.float32)
    cb[:, :OT] = bias.reshape(OT, 128).T
    cb[:, OT:] = alpha[None, :]

    return [
        {"xq4": xq4[q], "w5d": w5d, "cb": cb}
        for q in range(N_CORES)
    ]


def _ensure_trace_hook():
    """Make trace capture survive images whose antenv lacks axon_hooks.

    concourse.bass_utils imports antenv.axon_hooks unconditionally when
    tracing under axon; on images where trn_boot degraded (no
    axon_hooks module), that import crashes.  Install the same
    ctypes-based NTFF hook trn_boot would have registered.  No-op when
    the real module exists; never raises.
    """
    try:
        import antenv.axon_hooks  # noqa: F401

        return
    except Exception:
        pass
    try:
        import sys
        import types

        from trn_agent_boot.trn_boot import _ntff_profile_via_ctypes

        hook = _ntff_profile_via_ctypes("/opt/axon/libaxon_pjrt.so")
        mod = types.ModuleType("antenv.axon_hooks")
        mod.get_axon_ntff_profile_hook = lambda: hook
        mod.set_axon_ntff_profile_hook = lambda h: None
        sys.modules["antenv.axon_hooks"] = mod

        import concourse.bass_utils as bu

        orig_upload = bu.upload_artifacts

        def _safe_upload(tmpdir):
            try:
                return orig_upload(tmpdir)
            except Exception:
                return tmpdir

        bu.upload_artifacts = _safe_upload
    except Exception:
        pass


def kernel(x, w, bias, in_scale, w_scales):
    _ensure_trace_hook()
    nc = _build()
    in_maps = prepare_in_maps(x, w, bias, in_scale, w_scales)
    trace = bool(int(os.environ.get("TRN_KERNEL_TRACE", "0")))
    res = run_bass_kernel_spmd(
        nc,
        in_maps,
        list(range(N_CORES)),
        trace=trace,
        tmpdir=os.environ.get("TRN_KERNEL_TMPDIR") or None,
    )
    _CACHE["last_results"] = res

    out2d = np.empty((TOK, OUT), dtype=np.float32)
    for cid in range(N_CORES):
        out2d[T * cid : T * (cid + 1), :] = res.results[cid]["outT"].T.astype(
            np.float32
        )
    return out2d.reshape(B, S, OUT)


# revision 42
# speedup vs baseline: 1.0026x; 1.0026x over previous
"""Bass/Trainium2 kernel for nn_DefaultSegmentLinear (fp8 segment linear).

Reference semantics (CHUNKS=4, seg_mode='weight'):
    xq = e4m3fn(x / in_scale)                       # OCP e4m3, max 448
    wq = e4m3fn(w_c / w_scales[c])                  # per out-chunk of 1024
    out = (xq @ wq_c^T) * in_scale * w_scales[c] + bias

Sharding: 8-way over the 16384 tokens (each core owns 2048 tokens and
the full 4096 out features).  Per-core HBM traffic is then 8 MiB of
fp8 x + 16 MiB of fp8 w + 32 MiB of f32 out, far under the tensor
engine's ~445 us of fp8 matmul work, so the kernel is compute-bound.

Quantization runs on the HOST: x and w are divided by their
calibration scales (exact f32 division, matching the reference),
clipped to +-448, halved, and rounded to TRN e4m3 (IEEE-style, max
240).  Every OCP-e4m3 grid point v <= 448 has v/2 exactly
representable in TRN e4m3 (up to deep subnormals), and
round-to-nearest commutes with the exact *0.5, so the device sees
exactly the reference quantization grid at half scale.  The 4x is
folded into the output scale alpha_c = 4*in_scale*w_scales[c].
The device runs pure double-pumped fp8 matmuls (perf_mode=DoubleRow,
K=256 per instruction) with no on-device quantization pass at all.

Per-core tensors (contraction i on partitions for both operands):
    xq4  [16, 128, 2, 2048] fp8  pre-tiled (x/in_scale/2)^T so each
         k-supertile DMA is one 4 KiB contiguous line per partition
    w5d  [32, 128, 16, 2, 128] fp8  pre-tiled (w/w_scale/2)^T so each
         (o-tile, partition) reads 4 KiB contiguous
    outT [4096, 2048] f32  (o, t); host transposes back

PSUM tile [o=128, t=512]; per o-tile: 16 k-steps x 4 t-banks of
DoubleRow matmuls, then one DVE tensor_scalar (psum*alpha + bias) per
bank and a DMA out.  The first two o-tiles interleave their k-loops
(8 matmuls per arriving x k-tile) so the tensor engine keeps pace
with the initial x DMA stream instead of idling at startup; weights
for o-tile n+2 prefetch while n runs.
"""

import os

import ml_dtypes
import numpy as np

import concourse.bacc as bacc
import concourse.mybir as mybir
from concourse import tile
from concourse.bass_utils import run_bass_kernel_spmd

N_CORES = 8
B, S, IN, OUT = 4, 4096, 4096, 4096
TOK = B * S
T = TOK // N_CORES       # 2048 tokens per core
KT = IN // 256           # 16 contraction super-tiles (256 = 128 x 2)
OT = OUT // 128          # 32 out-feature tiles per core
NT = 512                 # moving free dim per matmul (one PSUM bank of f32)
TT = T // NT             # 4 token banks
CHUNKS = 4
OT_PER_CHUNK = OT // CHUNKS  # 8

F32 = mybir.dt.float32
BF16 = mybir.dt.bfloat16
FP8 = mybir.dt.float8e4
E4M3_MAX = 448.0

_CACHE = {}


def _build():
    if "nc" in _CACHE:
        return _CACHE["nc"]
    nc = bacc.Bacc(None, target_bir_lowering=False)
    # x streams as 512 KiB k-tiles with 4 KiB per-partition rows -- the
    # only DMA shape that runs at full ring rate (~386 GB/s; 2 KiB rows
    # measured ~145 GB/s, 1 KiB ~70).  x, the startup weights (wq0/wq1)
    # and the output writes ride the SP HWDGE ring; consts + remaining
    # weights ride the Activation ring, which only sustains ~160 GB/s
    # while SP is busy (HWDGE is FIFO per issuing engine).
    xq4 = nc.dram_tensor("xq4", [KT, 128, 2, T], FP8, kind="ExternalInput")
    w5d = nc.dram_tensor("w5d", [OT, 128, KT, 2, 128], FP8, kind="ExternalInput")
    # cb[p, j] = bias[128*j + p] for j < OT; cb[p, OT+c] = alpha[c].
    # Pre-tiled on host so the whole const set is ONE contiguous DMA --
    # per-element gathers here put ~5k 4-byte packets ahead of the first
    # weight/x tiles on the DMA queue and stall the PE for ~12 us.
    cb = nc.dram_tensor("cb", [128, OT + CHUNKS], F32, kind="ExternalInput")
    outT = nc.dram_tensor("outT", [OUT, T], BF16, kind="ExternalOutput")

    DR = mybir.MatmulPerfMode.DoubleRow

    with tile.TileContext(nc) as tc:
        with (
            tc.tile_pool(name="consts", bufs=1) as consts,
            tc.tile_pool(name="xq", bufs=1) as xqp,
            tc.tile_pool(name="wq", bufs=4) as wqp,
            tc.tile_pool(name="osb", bufs=4) as osbp,
            tc.tile_pool(name="psum", bufs=8, space="PSUM") as psp,
        ):
            wq_t = {}

            def load_wq(ot, eng):
                t = wqp.tile([128, KT, 2, 128], FP8, tag="wq", name=f"wq{ot}")
                eng.dma_start(out=t[:], in_=w5d[ot])
                wq_t[ot] = t

            # Each dma_start carries ~2 us of completion latency on top of
            # its transfer time, so the two first-matmul dependencies (xq0,
            # wq0) must ride DIFFERENT rings in parallel, each as its
            # ring's first item: x (then the output writes) on SP, weights
            # + consts on ACT.  Steady-state weight prefetches stay on ACT,
            # paced by wq-pool buffer reuse.
            xq = []

            def load_xq(k):
                xq_k = xqp.tile([128, 2, T], FP8, tag=f"xq{k}", name=f"xq{k}")
                nc.sync.dma_start(out=xq_k[:], in_=xq4[k])
                xq.append(xq_k)

            load_wq(0, nc.scalar)
            for k in range(KT):
                load_xq(k)
            load_wq(1, nc.scalar)
            cb_sb = consts.tile([128, OT + CHUNKS], F32, tag="cb")
            nc.scalar.dma_start(out=cb_sb[:], in_=cb[:])
            load_wq(2, nc.scalar)
            load_wq(3, nc.scalar)

            def rhs(k, tb):
                return xq[k][:, :, NT * tb : NT * (tb + 1)]

            def mms(wq, ps, k):
                for tb in range(TT):
                    nc.tensor.matmul(
                        ps[tb][:],
                        lhsT=wq[:, k, :, :],
                        rhs=rhs(k, tb),
                        start=(k == 0),
                        stop=(k == KT - 1),
                        perf_mode=DR,
                    )

            # One [128, T] bf16 out tile per o-tile: 4 banks of DVE
            # scale+bias land in its columns, then a single DMA with 4 KiB
            # rows writes it out (small-row DMAs run far below ring rate).
            def epilogue(ot, ps):
                c = ot // OT_PER_CHUNK
                ob = osbp.tile([128, T], BF16, tag="osb", name=f"ob{ot}")
                for tb in range(TT):
                    nc.vector.tensor_scalar(
                        ob[:, NT * tb : NT * (tb + 1)],
                        ps[tb][:],
                        cb_sb[:, OT + c : OT + c + 1],
                        cb_sb[:, ot : ot + 1],
                        op0=mybir.AluOpType.mult,
                        op1=mybir.AluOpType.add,
                    )
                nc.sync.dma_start(
                    out=outT[128 * ot : 128 * (ot + 1), :], in_=ob[:]
                )

            # ---- startup: o-tiles 0,1 with k-interleaved loops, so each
            # arriving xq[k] feeds 8 matmuls and the PE tracks the DMA
            # frontier ----
            psA = [
                [
                    psp.tile([128, NT], F32, tag="ps", name=f"ps{ot}_{tb}")
                    for tb in range(TT)
                ]
                for ot in range(2)
            ]
            # ot0 runs k0..k2 solo first: at ramp clock its consumption
            # matches the x delivery rate, and it buys time for wq1 to
            # arrive on the ACT ring without stalling the PE.  ot1 then
            # trails ot0 by a constant 3-k lag (each loop step consumes
            # exactly one new x tile), and ot0's epilogue overlaps ot1's
            # tail k-steps.
            SOLO = 3
            for k in range(SOLO):
                mms(wq_t[0], psA[0], k)
            for j in range(SOLO, KT):
                mms(wq_t[0], psA[0], j)
                mms(wq_t[1], psA[1], j - SOLO)
            epilogue(0, psA[0])
            for j in range(KT - SOLO, KT):
                mms(wq_t[1], psA[1], j)
            epilogue(1, psA[1])

            # ---- steady state: stream o-tiles 2..31, prefetch depth 2 ----
            for ot in range(2, OT - 1):
                ps = [
                    psp.tile([128, NT], F32, tag="ps", name=f"ps{ot}_{tb}")
                    for tb in range(TT)
                ]
                for k in range(KT):
                    mms(wq_t[ot], ps, k)
                if ot + 2 < OT:
                    load_wq(ot + 2, nc.scalar)
                epilogue(ot, ps)

            # ---- last o-tile: bank-by-bank (k inner) so each bank's
            # epilogue overlaps the remaining banks' matmuls; the output
            # leaves as two independent half-tiles so the left half's
            # drain hides under banks 2-3's matmuls and only ~256 KiB
            # remains after the final bank ----
            ot = OT - 1
            c = ot // OT_PER_CHUNK
            obH = [
                osbp.tile([128, 2 * NT], BF16, tag="osb", name=f"ob{ot}h{h}")
                for h in range(2)
            ]
            for tb in range(TT):
                ps_tb = psp.tile([128, NT], F32, tag="ps", name=f"ps{ot}_{tb}")
                for k in range(KT):
                    nc.tensor.matmul(
                        ps_tb[:],
                        lhsT=wq_t[ot][:, k, :, :],
                        rhs=rhs(k, tb),
                        start=(k == 0),
                        stop=(k == KT - 1),
                        perf_mode=DR,
                    )
                nc.vector.tensor_scalar(
                    obH[tb // 2][:, NT * (tb % 2) : NT * (tb % 2 + 1)],
                    ps_tb[:],
                    cb_sb[:, OT + c : OT + c + 1],
                    cb_sb[:, ot : ot + 1],
                    op0=mybir.AluOpType.mult,
                    op1=mybir.AluOpType.add,
                )
                if tb == 1:
                    nc.sync.dma_start(
                        out=outT[128 * ot : 128 * (ot + 1), 0 : 2 * NT],
                        in_=obH[0][:],
                    )
            nc.sync.dma_start(
                out=outT[128 * ot : 128 * (ot + 1), 2 * NT :], in_=obH[1][:]
            )
    nc.compile()
    _CACHE["nc"] = nc
    return nc


def prepare_in_maps(x, w, bias, in_scale, w_scales):
    """Host-side prep: scale-normalize, quantize to TRN e4m3, tile.

    Matches the reference grid: v = x/in_scale (exact f32 division),
    clip to +-448, then round-to-nearest onto the OCP e4m3 grid -- done
    here as round(v*0.5) onto the TRN e4m3 grid, identical because the
    grids coincide under the exact *0.5 (up to deep subnormals below
    2^-9, which are noise at this problem's scale).
    """
    assert x.shape == (B, S, IN) and w.shape == (OUT, IN)
    x = np.ascontiguousarray(x, dtype=np.float32)
    w = np.ascontiguousarray(w, dtype=np.float32)
    bias = np.ascontiguousarray(bias, dtype=np.float32)
    in_scale = np.float32(np.asarray(in_scale).reshape(()))
    w_scales = np.asarray(w_scales, dtype=np.float32).reshape(CHUNKS)

    t = x.reshape(TOK, IN) / in_scale
    np.clip(t, -E4M3_MAX, E4M3_MAX, out=t)
    t *= np.float32(0.5)
    xq8 = t.astype(ml_dtypes.float8_e4m3)
    # xq4[q][k, p, j, t'] = xq8[t = T*q + t', i = 256k + 128j + p]
    xq4 = np.ascontiguousarray(
        xq8.reshape(N_CORES, T, KT, 2, 128).transpose(0, 2, 4, 3, 1)
    )

    wn = w.reshape(CHUNKS, OUT // CHUNKS, IN) / w_scales[:, None, None]
    np.clip(wn, -E4M3_MAX, E4M3_MAX, out=wn)
    wn *= np.float32(0.5)
    wq8 = wn.reshape(OUT, IN).astype(ml_dtypes.float8_e4m3)
    # w5d[ot, p, k, j, m] = wq8[o = 128*ot + m, i = 256k + 128j + p]
    w5d = np.ascontiguousarray(
        wq8.T.reshape(KT, 2, 128, OT, 128).transpose(3, 2, 0, 1, 4)
    )

    alpha = (
        4.0 * in_scale.astype(np.float64) * w_scales.astype(np.float64)
    ).astype(np.float32)
    # cb[p, j<OT] = bias[128*j + p]; cb[p, OT+c] = alpha[c]
    cb = np.empty((128, OT + CHUNKS), dtype=np.float32)
    cb[:, :OT] = bias.reshape(OT, 128).T
    cb[:, OT:] = alpha[None, :]

    return [
        {"xq4": xq4[q], "w5d": w5d, "cb": cb}
        for q in range(N_CORES)
    ]


def _ensure_trace_hook():
    """Make trace capture survive images whose antenv lacks axon_hooks.

    concourse.bass_utils imports antenv.axon_hooks unconditionally when
    tracing under axon; on images where trn_boot degraded (no
    axon_hooks module), that import crashes.  Install the same
    ctypes-based NTFF hook trn_boot would have registered.  No-op when
    the real module exists; never raises.
    """
    try:
        import antenv.axon_hooks  # noqa: F401

        return
    except Exception:
        pass
    try:
        import sys
        import types

        from trn_agent_boot.trn_boot import _ntff_profile_via_ctypes

        hook = _ntff_profile_via_ctypes("/opt/axon/libaxon_pjrt.so")
        mod = types.ModuleType("antenv.axon_hooks")
        mod.get_axon_ntff_profile_hook = lambda: hook
        mod.set_axon_ntff_profile_hook = lambda h: None
        sys.modules["antenv.axon_hooks"] = mod

        import concourse.bass_utils as bu

        orig_upload = bu.upload_artifacts

        def _safe_upload(tmpdir):
            try:
                return orig_upload(tmpdir)
            except Exception:
                return tmpdir

        bu.upload_artifacts = _safe_upload
    except Exception:
        pass


def kernel(x, w, bias, in_scale, w_scales):
    _ensure_trace_hook()
    nc = _build()
    in_maps = prepare_in_maps(x, w, bias, in_scale, w_scales)
    trace = bool(int(os.environ.get("TRN_KERNEL_TRACE", "0")))
    res = run_bass_kernel_spmd(
        nc,
        in_maps,
        list(range(N_CORES)),
        trace=trace,
        tmpdir=os.environ.get("TRN_KERNEL_TMPDIR") or None,
    )
    _CACHE["last_results"] = res

    out2d = np.empty((TOK, OUT), dtype=np.float32)
    for cid in range(N_CORES):
        out2d[T * cid : T * (cid + 1), :] = res.results[cid]["outT"].T.astype(
            np.float32
        )
    return out2d.reshape(B, S, OUT)


# revision 43
# speedup vs baseline: 1.0027x; 1.0001x over previous
"""Bass/Trainium2 kernel for nn_DefaultSegmentLinear (fp8 segment linear).

Reference semantics (CHUNKS=4, seg_mode='weight'):
    xq = e4m3fn(x / in_scale)                       # OCP e4m3, max 448
    wq = e4m3fn(w_c / w_scales[c])                  # per out-chunk of 1024
    out = (xq @ wq_c^T) * in_scale * w_scales[c] + bias

Sharding: 8-way over the 16384 tokens (each core owns 2048 tokens and
the full 4096 out features).  Per-core HBM traffic is then 8 MiB of
fp8 x + 16 MiB of fp8 w + 32 MiB of f32 out, far under the tensor
engine's ~445 us of fp8 matmul work, so the kernel is compute-bound.

Quantization runs on the HOST: x and w are divided by their
calibration scales (exact f32 division, matching the reference),
clipped to +-448, halved, and rounded to TRN e4m3 (IEEE-style, max
240).  Every OCP-e4m3 grid point v <= 448 has v/2 exactly
representable in TRN e4m3 (up to deep subnormals), and
round-to-nearest commutes with the exact *0.5, so the device sees
exactly the reference quantization grid at half scale.  The 4x is
folded into the output scale alpha_c = 4*in_scale*w_scales[c].
The device runs pure double-pumped fp8 matmuls (perf_mode=DoubleRow,
K=256 per instruction) with no on-device quantization pass at all.

Per-core tensors (contraction i on partitions for both operands):
    xq4  [16, 128, 2, 2048] fp8  pre-tiled (x/in_scale/2)^T so each
         k-supertile DMA is one 4 KiB contiguous line per partition
    w5d  [32, 128, 16, 2, 128] fp8  pre-tiled (w/w_scale/2)^T so each
         (o-tile, partition) reads 4 KiB contiguous
    outT [4096, 2048] f32  (o, t); host transposes back

PSUM tile [o=128, t=512]; per o-tile: 16 k-steps x 4 t-banks of
DoubleRow matmuls, then one DVE tensor_scalar (psum*alpha + bias) per
bank and a DMA out.  The first two o-tiles interleave their k-loops
(8 matmuls per arriving x k-tile) so the tensor engine keeps pace
with the initial x DMA stream instead of idling at startup; weights
for o-tile n+2 prefetch while n runs.
"""

import os

import ml_dtypes
import numpy as np

import concourse.bacc as bacc
import concourse.mybir as mybir
from concourse import tile
from concourse.bass_utils import run_bass_kernel_spmd

N_CORES = 8
B, S, IN, OUT = 4, 4096, 4096, 4096
TOK = B * S
T = TOK // N_CORES       # 2048 tokens per core
KT = IN // 256           # 16 contraction super-tiles (256 = 128 x 2)
OT = OUT // 128          # 32 out-feature tiles per core
NT = 512                 # moving free dim per matmul (one PSUM bank of f32)
TT = T // NT             # 4 token banks
CHUNKS = 4
OT_PER_CHUNK = OT // CHUNKS  # 8

F32 = mybir.dt.float32
BF16 = mybir.dt.bfloat16
FP8 = mybir.dt.float8e4
E4M3_MAX = 448.0

_CACHE = {}


def _build():
    if "nc" in _CACHE:
        return _CACHE["nc"]
    nc = bacc.Bacc(None, target_bir_lowering=False)
    # x streams as 512 KiB k-tiles with 4 KiB per-partition rows -- the
    # only DMA shape that runs at full ring rate (~386 GB/s; 2 KiB rows
    # measured ~145 GB/s, 1 KiB ~70).  x, the startup weights (wq0/wq1)
    # and the output writes ride the SP HWDGE ring; consts + remaining
    # weights ride the Activation ring, which only sustains ~160 GB/s
    # while SP is busy (HWDGE is FIFO per issuing engine).
    xq4 = nc.dram_tensor("xq4", [KT, 128, 2, T], FP8, kind="ExternalInput")
    w5d = nc.dram_tensor("w5d", [OT, 128, KT, 2, 128], FP8, kind="ExternalInput")
    # cb[p, j] = bias[128*j + p] for j < OT; cb[p, OT+c] = alpha[c].
    # Pre-tiled on host so the whole const set is ONE contiguous DMA --
    # per-element gathers here put ~5k 4-byte packets ahead of the first
    # weight/x tiles on the DMA queue and stall the PE for ~12 us.
    cb = nc.dram_tensor("cb", [128, OT + CHUNKS], F32, kind="ExternalInput")
    outT = nc.dram_tensor("outT", [OUT, T], BF16, kind="ExternalOutput")

    DR = mybir.MatmulPerfMode.DoubleRow

    with tile.TileContext(nc) as tc:
        with (
            tc.tile_pool(name="consts", bufs=1) as consts,
            tc.tile_pool(name="xq", bufs=1) as xqp,
            tc.tile_pool(name="wq", bufs=4) as wqp,
            tc.tile_pool(name="osb", bufs=4) as osbp,
            tc.tile_pool(name="psum", bufs=8, space="PSUM") as psp,
        ):
            wq_t = {}

            def load_wq(ot, eng):
                t = wqp.tile([128, KT, 2, 128], FP8, tag="wq", name=f"wq{ot}")
                eng.dma_start(out=t[:], in_=w5d[ot])
                wq_t[ot] = t

            # Each dma_start carries ~2 us of completion latency on top of
            # its transfer time, so the two first-matmul dependencies (xq0,
            # wq0) must ride DIFFERENT rings in parallel, each as its
            # ring's first item: x (then the output writes) on SP, weights
            # + consts on ACT.  Steady-state weight prefetches stay on ACT,
            # paced by wq-pool buffer reuse.
            xq = []

            def load_xq(k, eng):
                xq_k = xqp.tile([128, 2, T], FP8, tag=f"xq{k}", name=f"xq{k}")
                eng.dma_start(out=xq_k[:], in_=xq4[k])
                xq.append(xq_k)

            # xq1 rides the ACT ring behind wq0 so the first two k-tiles
            # arrive in parallel on the two rings -- xq1 otherwise lands
            # ~3 us after xq0 (per-transfer receipt) and stalls the ramp-
            # phase k1 step.  wq1 shifts later but is only needed at ot1's
            # lagged start.
            load_wq(0, nc.scalar)
            load_xq(0, nc.sync)
            load_xq(1, nc.scalar)
            for k in range(2, KT):
                load_xq(k, nc.sync)
            load_wq(1, nc.scalar)
            cb_sb = consts.tile([128, OT + CHUNKS], F32, tag="cb")
            nc.scalar.dma_start(out=cb_sb[:], in_=cb[:])
            load_wq(2, nc.scalar)
            load_wq(3, nc.scalar)

            def rhs(k, tb):
                return xq[k][:, :, NT * tb : NT * (tb + 1)]

            def mms(wq, ps, k):
                for tb in range(TT):
                    nc.tensor.matmul(
                        ps[tb][:],
                        lhsT=wq[:, k, :, :],
                        rhs=rhs(k, tb),
                        start=(k == 0),
                        stop=(k == KT - 1),
                        perf_mode=DR,
                    )

            # One [128, T] bf16 out tile per o-tile: 4 banks of DVE
            # scale+bias land in its columns, then a single DMA with 4 KiB
            # rows writes it out (small-row DMAs run far below ring rate).
            def epilogue(ot, ps):
                c = ot // OT_PER_CHUNK
                ob = osbp.tile([128, T], BF16, tag="osb", name=f"ob{ot}")
                for tb in range(TT):
                    nc.vector.tensor_scalar(
                        ob[:, NT * tb : NT * (tb + 1)],
                        ps[tb][:],
                        cb_sb[:, OT + c : OT + c + 1],
                        cb_sb[:, ot : ot + 1],
                        op0=mybir.AluOpType.mult,
                        op1=mybir.AluOpType.add,
                    )
                nc.sync.dma_start(
                    out=outT[128 * ot : 128 * (ot + 1), :], in_=ob[:]
                )

            # ---- startup: o-tiles 0,1 with k-interleaved loops, so each
            # arriving xq[k] feeds 8 matmuls and the PE tracks the DMA
            # frontier ----
            psA = [
                [
                    psp.tile([128, NT], F32, tag="ps", name=f"ps{ot}_{tb}")
                    for tb in range(TT)
                ]
                for ot in range(2)
            ]
            # ot0 runs k0..k2 solo first: at ramp clock its consumption
            # matches the x delivery rate, and it buys time for wq1 to
            # arrive on the ACT ring without stalling the PE.  ot1 then
            # trails ot0 by a constant 3-k lag (each loop step consumes
            # exactly one new x tile), and ot0's epilogue overlaps ot1's
            # tail k-steps.
            SOLO = 3
            for k in range(SOLO):
                mms(wq_t[0], psA[0], k)
            for j in range(SOLO, KT):
                mms(wq_t[0], psA[0], j)
                mms(wq_t[1], psA[1], j - SOLO)
            epilogue(0, psA[0])
            for j in range(KT - SOLO, KT):
                mms(wq_t[1], psA[1], j)
            epilogue(1, psA[1])

            # ---- steady state: stream o-tiles 2..31, prefetch depth 2 ----
            for ot in range(2, OT - 1):
                ps = [
                    psp.tile([128, NT], F32, tag="ps", name=f"ps{ot}_{tb}")
                    for tb in range(TT)
                ]
                for k in range(KT):
                    mms(wq_t[ot], ps, k)
                if ot + 2 < OT:
                    load_wq(ot + 2, nc.scalar)
                epilogue(ot, ps)

            # ---- last o-tile: bank-by-bank (k inner) so each bank's
            # epilogue overlaps the remaining banks' matmuls; the output
            # leaves as two independent half-tiles so the left half's
            # drain hides under banks 2-3's matmuls and only ~256 KiB
            # remains after the final bank ----
            ot = OT - 1
            c = ot // OT_PER_CHUNK
            obH = [
                osbp.tile([128, 2 * NT], BF16, tag="osb", name=f"ob{ot}h{h}")
                for h in range(2)
            ]
            for tb in range(TT):
                ps_tb = psp.tile([128, NT], F32, tag="ps", name=f"ps{ot}_{tb}")
                for k in range(KT):
                    nc.tensor.matmul(
                        ps_tb[:],
                        lhsT=wq_t[ot][:, k, :, :],
                        rhs=rhs(k, tb),
                        start=(k == 0),
                        stop=(k == KT - 1),
                        perf_mode=DR,
                    )
                nc.vector.tensor_scalar(
                    obH[tb // 2][:, NT * (tb % 2) : NT * (tb % 2 + 1)],
                    ps_tb[:],
                    cb_sb[:, OT + c : OT + c + 1],
                    cb_sb[:, ot : ot + 1],
                    op0=mybir.AluOpType.mult,
                    op1=mybir.AluOpType.add,
                )
                if tb == 1:
                    nc.sync.dma_start(
                        out=outT[128 * ot : 128 * (ot + 1), 0 : 2 * NT],
                        in_=obH[0][:],
                    )
            nc.sync.dma_start(
                out=outT[128 * ot : 128 * (ot + 1), 2 * NT :], in_=obH[1][:]
            )
    nc.compile()
    _CACHE["nc"] = nc
    return nc


def prepare_in_maps(x, w, bias, in_scale, w_scales):
    """Host-side prep: scale-normalize, quantize to TRN e4m3, tile.

    Matches the reference grid: v = x/in_scale (exact f32 division),
    clip to +-448, then round-to-nearest onto the OCP e4m3 grid -- done
    here as round(v*0.5) onto the TRN e4m3 grid, identical because the
    grids coincide under the exact *0.5 (up to deep subnormals below
    2^-9, which are noise at this problem's scale).
    """
    assert x.shape == (B, S, IN) and w.shape == (OUT, IN)
    x = np.ascontiguousarray(x, dtype=np.float32)
    w = np.ascontiguousarray(w, dtype=np.float32)
    bias = np.ascontiguousarray(bias, dtype=np.float32)
    in_scale = np.float32(np.asarray(in_scale).reshape(()))
    w_scales = np.asarray(w_scales, dtype=np.float32).reshape(CHUNKS)

    t = x.reshape(TOK, IN) / in_scale
    np.clip(t, -E4M3_MAX, E4M3_MAX, out=t)
    t *= np.float32(0.5)
    xq8 = t.astype(ml_dtypes.float8_e4m3)
    # xq4[q][k, p, j, t'] = xq8[t = T*q + t', i = 256k + 128j + p]
    xq4 = np.ascontiguousarray(
        xq8.reshape(N_CORES, T, KT, 2, 128).transpose(0, 2, 4, 3, 1)
    )

    wn = w.reshape(CHUNKS, OUT // CHUNKS, IN) / w_scales[:, None, None]
    np.clip(wn, -E4M3_MAX, E4M3_MAX, out=wn)
    wn *= np.float32(0.5)
    wq8 = wn.reshape(OUT, IN).astype(ml_dtypes.float8_e4m3)
    # w5d[ot, p, k, j, m] = wq8[o = 128*ot + m, i = 256k + 128j + p]
    w5d = np.ascontiguousarray(
        wq8.T.reshape(KT, 2, 128, OT, 128).transpose(3, 2, 0, 1, 4)
    )

    alpha = (
        4.0 * in_scale.astype(np.float64) * w_scales.astype(np.float64)
    ).astype(np.float32)
    # cb[p, j<OT] = bias[128*j + p]; cb[p, OT+c] = alpha[c]
    cb = np.empty((128, OT + CHUNKS), dtype=np.float32)
    cb[:, :OT] = bias.reshape(OT, 128).T
    cb[:, OT:] = alpha[None, :]

    return [
        {"xq4": xq4[q], "w5d": w5d, "cb": cb}
        for q in range(N_CORES)
    ]


def _ensure_trace_hook():
    """Make trace capture survive images whose antenv lacks axon_hooks.

    concourse.bass_utils imports antenv.axon_hooks unconditionally when
    tracing under axon; on images where trn_boot degraded (no
    axon_hooks module), that import crashes.  Install the same
    ctypes-based NTFF hook trn_boot would have registered.  No-op when
    the real module exists; never raises.
    """
    try:
        import antenv.axon_hooks  # noqa: F401

        return
    except Exception:
        pass
    try:
        import sys
        import types

        from trn_agent_boot.trn_boot import _ntff_profile_via_ctypes

        hook = _ntff_profile_via_ctypes("/opt/axon/libaxon_pjrt.so")
        mod = types.ModuleType("antenv.axon_hooks")
        mod.get_axon_ntff_profile_hook = lambda: hook
        mod.set_axon_ntff_profile_hook = lambda h: None
        sys.modules["antenv.axon_hooks"] = mod

        import concourse.bass_utils as bu

        orig_upload = bu.upload_artifacts

        def _safe_upload(tmpdir):
            try:
                return orig_upload(tmpdir)
            except Exception:
                return tmpdir

        bu.upload_artifacts = _safe_upload
    except Exception:
        pass


def kernel(x, w, bias, in_scale, w_scales):
    _ensure_trace_hook()
    nc = _build()
    in_maps = prepare_in_maps(x, w, bias, in_scale, w_scales)
    trace = bool(int(os.environ.get("TRN_KERNEL_TRACE", "0")))
    res = run_bass_kernel_spmd(
        nc,
        in_maps,
        list(range(N_CORES)),
        trace=trace,
        tmpdir=os.environ.get("TRN_KERNEL_TMPDIR") or None,
    )
    _CACHE["last_results"] = res

    out2d = np.empty((TOK, OUT), dtype=np.float32)
    for cid in range(N_CORES):
        out2d[T * cid : T * (cid + 1), :] = res.results[cid]["outT"].T.astype(
            np.float32
        )
    return out2d.reshape(B, S, OUT)


# revision 44
# speedup vs baseline: 1.0048x; 1.0021x over previous
"""Bass/Trainium2 kernel for nn_DefaultSegmentLinear (fp8 segment linear).

Reference semantics (CHUNKS=4, seg_mode='weight'):
    xq = e4m3fn(x / in_scale)                       # OCP e4m3, max 448
    wq = e4m3fn(w_c / w_scales[c])                  # per out-chunk of 1024
    out = (xq @ wq_c^T) * in_scale * w_scales[c] + bias

Sharding: 8-way over the 16384 tokens (each core owns 2048 tokens and
the full 4096 out features).  Per-core HBM traffic is then 8 MiB of
fp8 x + 16 MiB of fp8 w + 32 MiB of f32 out, far under the tensor
engine's ~445 us of fp8 matmul work, so the kernel is compute-bound.

Quantization runs on the HOST: x and w are divided by their
calibration scales (exact f32 division, matching the reference),
clipped to +-448, halved, and rounded to TRN e4m3 (IEEE-style, max
240).  Every OCP-e4m3 grid point v <= 448 has v/2 exactly
representable in TRN e4m3 (up to deep subnormals), and
round-to-nearest commutes with the exact *0.5, so the device sees
exactly the reference quantization grid at half scale.  The 4x is
folded into the output scale alpha_c = 4*in_scale*w_scales[c].
The device runs pure double-pumped fp8 matmuls (perf_mode=DoubleRow,
K=256 per instruction) with no on-device quantization pass at all.

Per-core tensors (contraction i on partitions for both operands):
    xq4  [16, 128, 2, 2048] fp8  pre-tiled (x/in_scale/2)^T so each
         k-supertile DMA is one 4 KiB contiguous line per partition
    w5d  [32, 128, 16, 2, 128] fp8  pre-tiled (w/w_scale/2)^T so each
         (o-tile, partition) reads 4 KiB contiguous
    outT [4096, 2048] f32  (o, t); host transposes back

PSUM tile [o=128, t=512]; per o-tile: 16 k-steps x 4 t-banks of
DoubleRow matmuls, then one DVE tensor_scalar (psum*alpha + bias) per
bank and a DMA out.  The first two o-tiles interleave their k-loops
(8 matmuls per arriving x k-tile) so the tensor engine keeps pace
with the initial x DMA stream instead of idling at startup; weights
for o-tile n+2 prefetch while n runs.
"""

import os

import ml_dtypes
import numpy as np

import concourse.bacc as bacc
import concourse.mybir as mybir
from concourse import tile
from concourse.bass_utils import run_bass_kernel_spmd

N_CORES = 8
B, S, IN, OUT = 4, 4096, 4096, 4096
TOK = B * S
T = TOK // N_CORES       # 2048 tokens per core
KT = IN // 256           # 16 contraction super-tiles (256 = 128 x 2)
OT = OUT // 128          # 32 out-feature tiles per core
NT = 512                 # moving free dim per matmul (one PSUM bank of f32)
TT = T // NT             # 4 token banks
CHUNKS = 4
OT_PER_CHUNK = OT // CHUNKS  # 8

F32 = mybir.dt.float32
BF16 = mybir.dt.bfloat16
FP8 = mybir.dt.float8e4
E4M3_MAX = 448.0

_CACHE = {}


def _build():
    if "nc" in _CACHE:
        return _CACHE["nc"]
    nc = bacc.Bacc(None, target_bir_lowering=False)
    # x streams as 512 KiB k-tiles with 4 KiB per-partition rows -- the
    # only DMA shape that runs at full ring rate (~386 GB/s; 2 KiB rows
    # measured ~145 GB/s, 1 KiB ~70).  x, the startup weights (wq0/wq1)
    # and the output writes ride the SP HWDGE ring; consts + remaining
    # weights ride the Activation ring, which only sustains ~160 GB/s
    # while SP is busy (HWDGE is FIFO per issuing engine).
    xq4 = nc.dram_tensor("xq4", [KT, 128, 2, T], FP8, kind="ExternalInput")
    w5d = nc.dram_tensor("w5d", [OT, 128, KT, 2, 128], FP8, kind="ExternalInput")
    # cb[p, j] = bias[128*j + p] for j < OT; cb[p, OT+c] = alpha[c].
    # Pre-tiled on host so the whole const set is ONE contiguous DMA --
    # per-element gathers here put ~5k 4-byte packets ahead of the first
    # weight/x tiles on the DMA queue and stall the PE for ~12 us.
    cb = nc.dram_tensor("cb", [128, OT + CHUNKS], F32, kind="ExternalInput")
    outT = nc.dram_tensor("outT", [OUT, T], BF16, kind="ExternalOutput")

    DR = mybir.MatmulPerfMode.DoubleRow

    with tile.TileContext(nc) as tc:
        with (
            tc.tile_pool(name="consts", bufs=1) as consts,
            tc.tile_pool(name="xq", bufs=1) as xqp,
            tc.tile_pool(name="wq", bufs=4) as wqp,
            tc.tile_pool(name="osb", bufs=4) as osbp,
            tc.tile_pool(name="psum", bufs=8, space="PSUM") as psp,
        ):
            wq_t = {}

            def load_wq(ot, eng):
                t = wqp.tile([128, KT, 2, 128], FP8, tag="wq", name=f"wq{ot}")
                eng.dma_start(out=t[:], in_=w5d[ot])
                wq_t[ot] = t

            # Each dma_start carries ~2 us of completion latency on top of
            # its transfer time, so the two first-matmul dependencies (xq0,
            # wq0) must ride DIFFERENT rings in parallel, each as its
            # ring's first item: x (then the output writes) on SP, weights
            # + consts on ACT.  Steady-state weight prefetches stay on ACT,
            # paced by wq-pool buffer reuse.
            xq = []

            def load_xq(k):
                xq_k = xqp.tile([128, 2, T], FP8, tag=f"xq{k}", name=f"xq{k}")
                nc.sync.dma_start(out=xq_k[:], in_=xq4[k])
                xq.append(xq_k)

            load_wq(0, nc.scalar)
            for k in range(KT):
                load_xq(k)
            load_wq(1, nc.scalar)
            cb_sb = consts.tile([128, OT + CHUNKS], F32, tag="cb")
            nc.scalar.dma_start(out=cb_sb[:], in_=cb[:])
            load_wq(2, nc.scalar)
            load_wq(3, nc.scalar)

            def rhs(k, tb):
                return xq[k][:, :, NT * tb : NT * (tb + 1)]

            def mms(wq, ps, k):
                for tb in range(TT):
                    nc.tensor.matmul(
                        ps[tb][:],
                        lhsT=wq[:, k, :, :],
                        rhs=rhs(k, tb),
                        start=(k == 0),
                        stop=(k == KT - 1),
                        perf_mode=DR,
                    )

            # One [128, T] bf16 out tile per o-tile: 4 banks of DVE
            # scale+bias land in its columns, then a single DMA with 4 KiB
            # rows writes it out (small-row DMAs run far below ring rate).
            def epilogue(ot, ps):
                c = ot // OT_PER_CHUNK
                ob = osbp.tile([128, T], BF16, tag="osb", name=f"ob{ot}")
                for tb in range(TT):
                    nc.vector.tensor_scalar(
                        ob[:, NT * tb : NT * (tb + 1)],
                        ps[tb][:],
                        cb_sb[:, OT + c : OT + c + 1],
                        cb_sb[:, ot : ot + 1],
                        op0=mybir.AluOpType.mult,
                        op1=mybir.AluOpType.add,
                    )
                nc.sync.dma_start(
                    out=outT[128 * ot : 128 * (ot + 1), :], in_=ob[:]
                )

            # ---- startup: o-tiles 0,1 with k-interleaved loops, so each
            # arriving xq[k] feeds 8 matmuls and the PE tracks the DMA
            # frontier ----
            psA = [
                [
                    psp.tile([128, NT], F32, tag="ps", name=f"ps{ot}_{tb}")
                    for tb in range(TT)
                ]
                for ot in range(2)
            ]
            # ot0 runs k0..k2 solo first: at ramp clock its consumption
            # matches the x delivery rate, and it buys time for wq1 to
            # arrive on the ACT ring without stalling the PE.  ot1 then
            # trails ot0 by a constant 3-k lag (each loop step consumes
            # exactly one new x tile), and ot0's epilogue overlaps ot1's
            # tail k-steps.
            SOLO = 3
            for k in range(SOLO):
                mms(wq_t[0], psA[0], k)
            for j in range(SOLO, KT):
                mms(wq_t[0], psA[0], j)
                mms(wq_t[1], psA[1], j - SOLO)
            epilogue(0, psA[0])
            for j in range(KT - SOLO, KT):
                mms(wq_t[1], psA[1], j)
            epilogue(1, psA[1])

            # ---- steady state: stream o-tiles 2..31, prefetch depth 2 ----
            for ot in range(2, OT - 1):
                ps = [
                    psp.tile([128, NT], F32, tag="ps", name=f"ps{ot}_{tb}")
                    for tb in range(TT)
                ]
                for k in range(KT):
                    mms(wq_t[ot], ps, k)
                if ot + 2 < OT:
                    load_wq(ot + 2, nc.scalar)
                epilogue(ot, ps)

            # ---- last o-tile: bank-by-bank (k inner) so each bank's
            # epilogue overlaps the remaining banks' matmuls; the output
            # leaves as two independent half-tiles so the left half's
            # drain hides under banks 2-3's matmuls and only ~256 KiB
            # remains after the final bank ----
            ot = OT - 1
            c = ot // OT_PER_CHUNK
            obH = [
                osbp.tile([128, 2 * NT], BF16, tag="osb", name=f"ob{ot}h{h}")
                for h in range(2)
            ]
            for tb in range(TT):
                ps_tb = psp.tile([128, NT], F32, tag="ps", name=f"ps{ot}_{tb}")
                for k in range(KT):
                    nc.tensor.matmul(
                        ps_tb[:],
                        lhsT=wq_t[ot][:, k, :, :],
                        rhs=rhs(k, tb),
                        start=(k == 0),
                        stop=(k == KT - 1),
                        perf_mode=DR,
                    )
                nc.vector.tensor_scalar(
                    obH[tb // 2][:, NT * (tb % 2) : NT * (tb % 2 + 1)],
                    ps_tb[:],
                    cb_sb[:, OT + c : OT + c + 1],
                    cb_sb[:, ot : ot + 1],
                    op0=mybir.AluOpType.mult,
                    op1=mybir.AluOpType.add,
                )
                if tb == 1:
                    nc.sync.dma_start(
                        out=outT[128 * ot : 128 * (ot + 1), 0 : 2 * NT],
                        in_=obH[0][:],
                    )
            nc.sync.dma_start(
                out=outT[128 * ot : 128 * (ot + 1), 2 * NT :], in_=obH[1][:]
            )
    nc.compile()
    _CACHE["nc"] = nc
    return nc


def prepare_in_maps(x, w, bias, in_scale, w_scales):
    """Host-side prep: scale-normalize, quantize to TRN e4m3, tile.

    Matches the reference grid: v = x/in_scale (exact f32 division),
    clip to +-448, then round-to-nearest onto the OCP e4m3 grid -- done
    here as round(v*0.5) onto the TRN e4m3 grid, identical because the
    grids coincide under the exact *0.5 (up to deep subnormals below
    2^-9, which are noise at this problem's scale).
    """
    assert x.shape == (B, S, IN) and w.shape == (OUT, IN)
    x = np.ascontiguousarray(x, dtype=np.float32)
    w = np.ascontiguousarray(w, dtype=np.float32)
    bias = np.ascontiguousarray(bias, dtype=np.float32)
    in_scale = np.float32(np.asarray(in_scale).reshape(()))
    w_scales = np.asarray(w_scales, dtype=np.float32).reshape(CHUNKS)

    t = x.reshape(TOK, IN) / in_scale
    np.clip(t, -E4M3_MAX, E4M3_MAX, out=t)
    t *= np.float32(0.5)
    xq8 = t.astype(ml_dtypes.float8_e4m3)
    # xq4[q][k, p, j, t'] = xq8[t = T*q + t', i = 256k + 128j + p]
    xq4 = np.ascontiguousarray(
        xq8.reshape(N_CORES, T, KT, 2, 128).transpose(0, 2, 4, 3, 1)
    )

    wn = w.reshape(CHUNKS, OUT // CHUNKS, IN) / w_scales[:, None, None]
    np.clip(wn, -E4M3_MAX, E4M3_MAX, out=wn)
    wn *= np.float32(0.5)
    wq8 = wn.reshape(OUT, IN).astype(ml_dtypes.float8_e4m3)
    # w5d[ot, p, k, j, m] = wq8[o = 128*ot + m, i = 256k + 128j + p]
    w5d = np.ascontiguousarray(
        wq8.T.reshape(KT, 2, 128, OT, 128).transpose(3, 2, 0, 1, 4)
    )

    alpha = (
        4.0 * in_scale.astype(np.float64) * w_scales.astype(np.float64)
    ).astype(np.float32)
    # cb[p, j<OT] = bias[128*j + p]; cb[p, OT+c] = alpha[c]
    cb = np.empty((128, OT + CHUNKS), dtype=np.float32)
    cb[:, :OT] = bias.reshape(OT, 128).T
    cb[:, OT:] = alpha[None, :]

    return [
        {"xq4": xq4[q], "w5d": w5d, "cb": cb}
        for q in range(N_CORES)
    ]


def _ensure_trace_hook():
    """Make trace capture survive images whose antenv lacks axon_hooks.

    concourse.bass_utils imports antenv.axon_hooks unconditionally when
    tracing under axon; on images where trn_boot degraded (no
    axon_hooks module), that import crashes.  Install the same
    ctypes-based NTFF hook trn_boot would have registered.  No-op when
    the real module exists; never raises.
    """
    try:
        import antenv.axon_hooks  # noqa: F401

        return
    except Exception:
        pass
    try:
        import sys
        import types

        from trn_agent_boot.trn_boot import _ntff_profile_via_ctypes

        hook = _ntff_profile_via_ctypes("/opt/axon/libaxon_pjrt.so")
        mod = types.ModuleType("antenv.axon_hooks")
        mod.get_axon_ntff_profile_hook = lambda: hook
        mod.set_axon_ntff_profile_hook = lambda h: None
        sys.modules["antenv.axon_hooks"] = mod

        import concourse.bass_utils as bu

        orig_upload = bu.upload_artifacts

        def _safe_upload(tmpdir):
            try:
                return orig_upload(tmpdir)
            except Exception:
                return tmpdir

        bu.upload_artifacts = _safe_upload
    except Exception:
        pass


def kernel(x, w, bias, in_scale, w_scales):
    _ensure_trace_hook()
    nc = _build()
    in_maps = prepare_in_maps(x, w, bias, in_scale, w_scales)
    trace = bool(int(os.environ.get("TRN_KERNEL_TRACE", "0")))
    res = run_bass_kernel_spmd(
        nc,
        in_maps,
        list(range(N_CORES)),
        trace=trace,
        tmpdir=os.environ.get("TRN_KERNEL_TMPDIR") or None,
    )
    _CACHE["last_results"] = res

    out2d = np.empty((TOK, OUT), dtype=np.float32)
    for cid in range(N_CORES):
        out2d[T * cid : T * (cid + 1), :] = res.results[cid]["outT"].T.astype(
            np.float32
        )
    return out2d.reshape(B, S, OUT)
